# revision 22
# baseline (speedup 1.0000x reference)
"""Contrastive pair loss on 8 Trainium2 NeuronCores.

loss = mean_b( relu(mean_i((z1[b,i]-z2[b,i])^2) - margin) )  for
z1, z2 of shape (1024, 256, 16, 16) fp32.

Sharding: data-parallel over the batch axis — each of the 8 cores gets 128
rows (one row = 65536 values). The kernel streams both shards through
SBUF in [128, F] tiles; DVE computes d = z1-z2 (bf16), ACT squares d with
per-partition accumulation (accum_out). Per-row sums are DMA'd out and
the hinge/mean epilogue over 1024 rows runs on host.

Precision/bandwidth split, from measured rates (ns per 128-lane column:
ACT activation 0.87 for any dtype; DVE sub 1.06 from fp8 inputs but 0.54
from bf16; DMA 0.356 per fp8 column-pair): the host casts the first
46080 columns of each row to fp8 e4m3 and the last 19456 to bf16. That
balances all three units at ~60us/core: DMA 60.4, DVE 59.4, ACT 60.1.
Quantization adds ~6e-4 relative bias to the loss — 30x inside the 2e-2
gate. fp8 and bf16 tiles are interleaved so every pipeline window stays
DMA-bound (fp8 tiles are DVE-heavy, bf16 tiles are DMA-heavy).

GPSIMD runs no math: its tensor ops are software loops on the Q7 cores
that also degrade any concurrently-starting DVE op ~2x (measured), so it
only issues the z2 DMA descriptors (second HWDGE ring, keeping z1 and z2
streams on separate rings). TensorE is idle: matmul reduces along
partitions, the row sums here are along the free axis.
"""

import numpy as np

B = 1024
CODE = 256 * 16 * 16  # 65536
N_CORES = 8
ROWS = B // N_CORES  # 128 rows per core == SBUF partition count

# (dtype, width) per DMA load; fp8 total 46080 cols, bf16 total 19456.
# A small first load starts compute early; 2-4MB body transfers keep the
# single DMA ring at its measured ~390 GB/s; small tail loads shrink the
# serial compute after the last load. Compute is chunked to <= 8192 cols
# per instruction regardless of load width (CHUNK below).
LOADS = [
    ("8", 4096), ("8", 16384), ("8", 16384), ("8", 9216),
    ("b", 8192), ("b", 8192), ("b", 2048), ("b", 1024),
]
CHUNK = 8192
TILES = [(t, min(CHUNK, f - o)) for t, f in LOADS for o in range(0, f, CHUNK)]
# The second-to-last bf16 8192 tile gets its square split ACT/DVE-stt
# down the middle: near the tail DVE is delivery-gated with slack while
# ACT would otherwise serialize the finish.
STT_SPLIT = {5: 4096}  # load index -> cols squared on ACT
C8 = sum(f for t, f in LOADS if t == "8")  # 46080
CB = sum(f for t, f in LOADS if t == "b")  # 19456
NT = len(TILES)
MARGIN = 0.01

_CACHE = {}


def _split_multi_waits(nc):
    """The walrus build in this image rejects instructions carrying more
    than one sync-wait command ("Too many sync wait commands",
    setupSyncWait). Tile routinely emits several waits on one instruction,
    so split them: for each instruction with N>1 waits, inject N-1
    single-wait NoOps on the same engine immediately before it. Same-engine
    program order makes this semantically identical."""
    from concourse import mybir

    k = 0
    for fn in nc.m.functions:
        for blk in fn.blocks:
            insts = blk.instructions
            out = []
            changed = False
            for ins in insts:
                si = ins.sync_info
                if si is not None and si.on_wait and len(si.on_wait) > 1:
                    waits = list(si.on_wait)
                    for w in waits[:-1]:
                        k += 1
                        nop = mybir.InstNoOp(
                            name=f"WSPLIT-{k}",
                            text_hint="split_wait",
                            bass_nofuse=True,
                        )
                        nop.engine = ins.engine
                        nop.sync_info = mybir.SyncInfo(on_wait=[w], on_update=[])
                        out.append(nop)
                    si.on_wait = waits[-1:]
                    ins.sync_info = si
                    changed = True
                out.append(ins)
            if changed:
                blk.instructions = out


def _patch_lean_epilogue():
    """Tile's kernel-tail epilogue is drain + EVSEM-butterfly barrier +
    sem clears + second butterfly. Replace the two full (drain+butterfly)
    barriers with sequencer-level sem-only barriers; DMA completion is
    already guaranteed by the drain's sem waits."""
    from concourse.tile import TileContext, ScopedClock

    if getattr(TileContext, "_ant_lean_epilogue", False):
        return

    def _drain_and_barrier(self, tick_clock, wait_clock):
        nc = self.nc
        drain_inst = nc.sync.drain()
        wait_clock.add_sem_waits(
            drain_inst.ins, ScopedClock({None: tick_clock.global_clock})
        )
        nc.all_engine_barrier(sem_only=True)
        assert self.sems is not None
        popped = nc._tile_sem_poison_stack.pop()
        assert popped is self._sem_poison
        nc.clear_and_free_semaphores(list(self.sems.allocated().values()))
        nc.all_engine_barrier(sem_only=True)

    TileContext._drain_and_barrier = _drain_and_barrier
    TileContext._ant_lean_epilogue = True


def _build():
    if "nc" in _CACHE:
        return _CACHE["nc"]

    import concourse.bass as bass
    from concourse import mybir
    from concourse.tile import TileContext

    _patch_lean_epilogue()

    nc = bass.Bass("TRN2", target_bir_lowering=False, num_devices=N_CORES)
    z18 = nc.dram_tensor("z18", [ROWS, C8], mybir.dt.float8e4, kind="ExternalInput")
    z28 = nc.dram_tensor("z28", [ROWS, C8], mybir.dt.float8e4, kind="ExternalInput")
    z1b = nc.dram_tensor("z1b", [ROWS, CB], mybir.dt.bfloat16, kind="ExternalInput")
    z2b = nc.dram_tensor("z2b", [ROWS, CB], mybir.dt.bfloat16, kind="ExternalInput")
    out = nc.dram_tensor("out", [ROWS, 1], mybir.dt.float32, kind="ExternalOutput")

    w8 = max(f for t, f in LOADS if t == "8")
    wb = max(f for t, f in LOADS if t == "b")

    with TileContext(nc) as tc:
        with (
            tc.tile_pool(name="z18p", bufs=3) as p18,
            tc.tile_pool(name="z28p", bufs=3) as p28,
            tc.tile_pool(name="z1bp", bufs=2) as p1b,
            tc.tile_pool(name="z2bp", bufs=2) as p2b,
            tc.tile_pool(name="dp", bufs=2) as pd,
            tc.tile_pool(name="st", bufs=1) as ps,
        ):
            acc = ps.tile([ROWS, 2 * NT], mybir.dt.float32)
            dummy = ps.tile([ROWS, 1], mybir.dt.float32)
            dumb = ps.tile([ROWS, 1], mybir.dt.bfloat16)
            nc.vector.memset(acc[:], 0.0)
            c8 = cb = 0
            j = 0
            for li, (t, f) in enumerate(LOADS):
                if t == "8":
                    t1 = p18.tile([ROWS, w8], mybir.dt.float8e4)
                    nc.sync.dma_start(out=t1[:, :f], in_=z18[:, c8 : c8 + f])
                    t2 = p28.tile([ROWS, w8], mybir.dt.float8e4)
                    nc.sync.dma_start(out=t2[:, :f], in_=z28[:, c8 : c8 + f])
                    c8 += f
                else:
                    t1 = p1b.tile([ROWS, wb], mybir.dt.bfloat16)
                    nc.sync.dma_start(out=t1[:, :f], in_=z1b[:, cb : cb + f])
                    t2 = p2b.tile([ROWS, wb], mybir.dt.bfloat16)
                    nc.sync.dma_start(out=t2[:, :f], in_=z2b[:, cb : cb + f])
                    cb += f
                for o in range(0, f, CHUNK):
                    w = min(CHUNK, f - o)
                    d = pd.tile([ROWS, CHUNK], mybir.dt.bfloat16)
                    nc.vector.tensor_sub(
                        out=d[:, :w], in0=t1[:, o : o + w], in1=t2[:, o : o + w]
                    )
                    aw = STT_SPLIT.get(li, w)
                    nc.scalar.activation(
                        out=dummy[:].broadcast_to((ROWS, aw)),
                        in_=d[:, :aw],
                        func=mybir.ActivationFunctionType.Square,
                        accum_out=acc[:, j : j + 1],
                    )
                    if aw < w:
                        nc.vector.scalar_tensor_tensor(
                            out=dumb[:].broadcast_to((ROWS, w - aw)),
                            in0=d[:, aw:w],
                            scalar=0.0,
                            in1=d[:, aw:w],
                            op0=mybir.AluOpType.bypass,
                            op1=mybir.AluOpType.mult,
                            accum_out=acc[:, NT + j : NT + j + 1],
                        )
                    j += 1
            # Warm-up transfer: the last compute d-tile is rewritten from
            # DRAM (WAR dep puts it right before the output DMA in ring
            # order) so the SDMA path is active when the 512B output
            # transfer lands — the final completion sem fires faster on a
            # busy ring than after ~10us of idle.
            nc.sync.dma_start(out=t1[:, :2048], in_=z1b[:, :2048])
            rowsum = ps.tile([ROWS, 1], mybir.dt.float32)
            nc.vector.tensor_reduce(
                out=rowsum[:],
                in_=acc[:],
                axis=mybir.AxisListType.X,
                op=mybir.AluOpType.add,
            )
            nc.sync.dma_start(out=out[:], in_=rowsum[:])

    _split_multi_waits(nc)

    _CACHE["nc"] = nc
    return nc


def _run(z1, z2, trace=False):
    import ml_dtypes
    from concourse.bass_utils import run_bass_kernel_spmd

    nc = _build()
    fp8 = ml_dtypes.float8_e4m3
    bf16 = ml_dtypes.bfloat16
    z1f = np.ascontiguousarray(np.asarray(z1, dtype=np.float32)).reshape(B, CODE)
    z2f = np.ascontiguousarray(np.asarray(z2, dtype=np.float32)).reshape(B, CODE)
    z18 = z1f[:, :C8].astype(fp8)
    z28 = z2f[:, :C8].astype(fp8)
    z1b = z1f[:, C8:].astype(bf16)
    z2b = z2f[:, C8:].astype(bf16)
    in_maps = [
        {
            "z18": z18[c * ROWS : (c + 1) * ROWS],
            "z28": z28[c * ROWS : (c + 1) * ROWS],
            "z1b": z1b[c * ROWS : (c + 1) * ROWS],
            "z2b": z2b[c * ROWS : (c + 1) * ROWS],
        }
        for c in range(N_CORES)
    ]
    res = run_bass_kernel_spmd(
        nc, in_maps, core_ids=list(range(N_CORES)), trace=trace
    )
    rowsum = np.concatenate(
        [res.results[c]["out"][:, 0] for c in range(N_CORES)]
    ).astype(np.float64)
    hamm = rowsum / CODE
    hinged = np.where(hamm > MARGIN, hamm - MARGIN, 0.0)
    loss = np.float32(hinged.sum() / B)
    return np.asarray(loss, dtype=np.float32), res


def kernel(z1, z2):
    return _run(z1, z2, trace=False)[0]


# revision 27
# speedup vs baseline: 1.0036x; 1.0036x over previous
"""Contrastive pair loss on 8 Trainium2 NeuronCores.

loss = mean_b( relu(mean_i((z1[b,i]-z2[b,i])^2) - margin) )  for
z1, z2 of shape (1024, 256, 16, 16) fp32.

Sharding: data-parallel over the batch axis — each of the 8 cores gets 128
rows (one row = 65536 values). The kernel streams both shards through
SBUF in [128, F] tiles; DVE computes d = z1-z2 (bf16), ACT squares d with
per-partition accumulation (accum_out). Per-row sums are DMA'd out and
the hinge/mean epilogue over 1024 rows runs on host.

Precision/bandwidth split, from measured rates (ns per 128-lane column:
ACT activation 0.87 for any dtype; DVE sub 1.06 from fp8 inputs but 0.54
from bf16; DMA 0.356 per fp8 column-pair): the host casts the first
46080 columns of each row to fp8 e4m3 and the last 19456 to bf16. That
balances all three units at ~60us/core: DMA 60.4, DVE 59.4, ACT 60.1.
Quantization adds ~6e-4 relative bias to the loss — 30x inside the 2e-2
gate. fp8 and bf16 tiles are interleaved so every pipeline window stays
DMA-bound (fp8 tiles are DVE-heavy, bf16 tiles are DMA-heavy).

GPSIMD runs no math: its tensor ops are software loops on the Q7 cores
that also degrade any concurrently-starting DVE op ~2x (measured), so it
only issues the z2 DMA descriptors (second HWDGE ring, keeping z1 and z2
streams on separate rings). TensorE is idle: matmul reduces along
partitions, the row sums here are along the free axis.
"""

import numpy as np

B = 1024
CODE = 256 * 16 * 16  # 65536
N_CORES = 8
ROWS = B // N_CORES  # 128 rows per core == SBUF partition count

# (dtype, width) per DMA load; fp8 total 46080 cols, bf16 total 19456.
# A small first load starts compute early; 2-4MB body transfers keep the
# single DMA ring at its measured ~390 GB/s; small tail loads shrink the
# serial compute after the last load. Compute is chunked to <= 8192 cols
# per instruction regardless of load width (CHUNK below).
LOADS = [
    ("8", 2048), ("8", 4096), ("8", 8192), ("8", 8192), ("8", 8192),
    ("8", 8192), ("8", 7168), ("b", 8192), ("b", 8192), ("b", 2048),
    ("b", 1024),
]
CHUNK = 8192
TILES = [(t, min(CHUNK, f - o)) for t, f in LOADS for o in range(0, f, CHUNK)]
# bf16 8192 tiles get their square split ACT/DVE-stt down the middle, and
# the last tiny tile squares fully on DVE: in the tail phase DVE is
# delivery-gated with slack while ACT would otherwise serialize the
# finish.
STT_SPLIT = {7: 4096, 8: 4096, 10: 0}  # load index -> cols squared on ACT
C8 = sum(f for t, f in LOADS if t == "8")  # 46080
CB = sum(f for t, f in LOADS if t == "b")  # 19456
NT = len(TILES)
MARGIN = 0.01

_CACHE = {}


def _split_multi_waits(nc):
    """The walrus build in this image rejects instructions carrying more
    than one sync-wait command ("Too many sync wait commands",
    setupSyncWait). Tile routinely emits several waits on one instruction,
    so split them: for each instruction with N>1 waits, inject N-1
    single-wait NoOps on the same engine immediately before it. Same-engine
    program order makes this semantically identical."""
    from concourse import mybir

    k = 0
    for fn in nc.m.functions:
        for blk in fn.blocks:
            insts = blk.instructions
            out = []
            changed = False
            for ins in insts:
                si = ins.sync_info
                if si is not None and si.on_wait and len(si.on_wait) > 1:
                    waits = list(si.on_wait)
                    for w in waits[:-1]:
                        k += 1
                        nop = mybir.InstNoOp(
                            name=f"WSPLIT-{k}",
                            text_hint="split_wait",
                            bass_nofuse=True,
                        )
                        nop.engine = ins.engine
                        nop.sync_info = mybir.SyncInfo(on_wait=[w], on_update=[])
                        out.append(nop)
                    si.on_wait = waits[-1:]
                    ins.sync_info = si
                    changed = True
                out.append(ins)
            if changed:
                blk.instructions = out


def _patch_lean_epilogue():
    """Tile's kernel-tail epilogue is drain + EVSEM-butterfly barrier +
    sem clears + second butterfly. Replace the two full (drain+butterfly)
    barriers with sequencer-level sem-only barriers; DMA completion is
    already guaranteed by the drain's sem waits."""
    from concourse.tile import TileContext, ScopedClock

    if getattr(TileContext, "_ant_lean_epilogue", False):
        return

    def _drain_and_barrier(self, tick_clock, wait_clock):
        nc = self.nc
        drain_inst = nc.sync.drain()
        wait_clock.add_sem_waits(
            drain_inst.ins, ScopedClock({None: tick_clock.global_clock})
        )
        nc.all_engine_barrier(sem_only=True)
        assert self.sems is not None
        popped = nc._tile_sem_poison_stack.pop()
        assert popped is self._sem_poison
        nc.clear_and_free_semaphores(list(self.sems.allocated().values()))
        nc.all_engine_barrier(sem_only=True)

    TileContext._drain_and_barrier = _drain_and_barrier
    TileContext._ant_lean_epilogue = True


def _build():
    if "nc" in _CACHE:
        return _CACHE["nc"]

    import concourse.bass as bass
    from concourse import mybir
    from concourse.tile import TileContext

    _patch_lean_epilogue()

    nc = bass.Bass("TRN2", target_bir_lowering=False, num_devices=N_CORES)
    z18 = nc.dram_tensor("z18", [ROWS, C8], mybir.dt.float8e4, kind="ExternalInput")
    z28 = nc.dram_tensor("z28", [ROWS, C8], mybir.dt.float8e4, kind="ExternalInput")
    z1b = nc.dram_tensor("z1b", [ROWS, CB], mybir.dt.bfloat16, kind="ExternalInput")
    z2b = nc.dram_tensor("z2b", [ROWS, CB], mybir.dt.bfloat16, kind="ExternalInput")
    out = nc.dram_tensor("out", [ROWS, 1], mybir.dt.float32, kind="ExternalOutput")

    w8 = max(f for t, f in LOADS if t == "8")
    wb = max(f for t, f in LOADS if t == "b")

    with TileContext(nc) as tc:
        with (
            tc.tile_pool(name="z18p", bufs=4) as p18,
            tc.tile_pool(name="z28p", bufs=4) as p28,
            tc.tile_pool(name="z1bp", bufs=2) as p1b,
            tc.tile_pool(name="z2bp", bufs=2) as p2b,
            tc.tile_pool(name="t1p", bufs=2) as pt1,
            tc.tile_pool(name="t2p", bufs=2) as pt2,
            tc.tile_pool(name="dp", bufs=3) as pd,
            tc.tile_pool(name="st", bufs=1) as ps,
        ):
            acc = ps.tile([ROWS, 2 * NT], mybir.dt.float32)
            dummy = ps.tile([ROWS, 1], mybir.dt.float32)
            dumb = ps.tile([ROWS, 1], mybir.dt.bfloat16)
            nc.vector.memset(acc[:], 0.0)
            c8 = cb = 0
            j = 0
            for li, (t, f) in enumerate(LOADS):
                if t == "8":
                    t1 = p18.tile([ROWS, w8], mybir.dt.float8e4)
                    nc.sync.dma_start(out=t1[:, :f], in_=z18[:, c8 : c8 + f])
                    t2 = p28.tile([ROWS, w8], mybir.dt.float8e4)
                    nc.sync.dma_start(out=t2[:, :f], in_=z28[:, c8 : c8 + f])
                    c8 += f
                else:
                    q1, q2, w_ = (p1b, p2b, wb) if f > 2048 else (pt1, pt2, 2048)
                    t1 = q1.tile([ROWS, w_], mybir.dt.bfloat16)
                    nc.sync.dma_start(out=t1[:, :f], in_=z1b[:, cb : cb + f])
                    t2 = q2.tile([ROWS, w_], mybir.dt.bfloat16)
                    nc.sync.dma_start(out=t2[:, :f], in_=z2b[:, cb : cb + f])
                    cb += f
                for o in range(0, f, CHUNK):
                    w = min(CHUNK, f - o)
                    d = pd.tile([ROWS, CHUNK], mybir.dt.bfloat16)
                    nc.vector.tensor_sub(
                        out=d[:, :w], in0=t1[:, o : o + w], in1=t2[:, o : o + w]
                    )
                    aw = STT_SPLIT.get(li, w)
                    if aw > 0:
                        nc.scalar.activation(
                            out=dummy[:].broadcast_to((ROWS, aw)),
                            in_=d[:, :aw],
                            func=mybir.ActivationFunctionType.Square,
                            accum_out=acc[:, j : j + 1],
                        )
                    if aw < w:
                        nc.vector.scalar_tensor_tensor(
                            out=dumb[:].broadcast_to((ROWS, w - aw)),
                            in0=d[:, aw:w],
                            scalar=0.0,
                            in1=d[:, aw:w],
                            op0=mybir.AluOpType.bypass,
                            op1=mybir.AluOpType.mult,
                            accum_out=acc[:, NT + j : NT + j + 1],
                        )
                    j += 1
            rowsum = ps.tile([ROWS, 1], mybir.dt.float32)
            nc.vector.tensor_reduce(
                out=rowsum[:],
                in_=acc[:],
                axis=mybir.AxisListType.X,
                op=mybir.AluOpType.add,
            )
            nc.sync.dma_start(out=out[:], in_=rowsum[:])

    _split_multi_waits(nc)

    _CACHE["nc"] = nc
    return nc


def _run(z1, z2, trace=False):
    import ml_dtypes
    from concourse.bass_utils import run_bass_kernel_spmd

    nc = _build()
    fp8 = ml_dtypes.float8_e4m3
    bf16 = ml_dtypes.bfloat16
    z1f = np.ascontiguousarray(np.asarray(z1, dtype=np.float32)).reshape(B, CODE)
    z2f = np.ascontiguousarray(np.asarray(z2, dtype=np.float32)).reshape(B, CODE)
    z18 = z1f[:, :C8].astype(fp8)
    z28 = z2f[:, :C8].astype(fp8)
    z1b = z1f[:, C8:].astype(bf16)
    z2b = z2f[:, C8:].astype(bf16)
    in_maps = [
        {
            "z18": z18[c * ROWS : (c + 1) * ROWS],
            "z28": z28[c * ROWS : (c + 1) * ROWS],
            "z1b": z1b[c * ROWS : (c + 1) * ROWS],
            "z2b": z2b[c * ROWS : (c + 1) * ROWS],
        }
        for c in range(N_CORES)
    ]
    res = run_bass_kernel_spmd(
        nc, in_maps, core_ids=list(range(N_CORES)), trace=trace
    )
    rowsum = np.concatenate(
        [res.results[c]["out"][:, 0] for c in range(N_CORES)]
    ).astype(np.float64)
    hamm = rowsum / CODE
    hinged = np.where(hamm > MARGIN, hamm - MARGIN, 0.0)
    loss = np.float32(hinged.sum() / B)
    return np.asarray(loss, dtype=np.float32), res


def kernel(z1, z2):
    return _run(z1, z2, trace=False)[0]
